# revision 4
# baseline (speedup 1.0000x reference)
"""Trainium2 Bass kernel for block-causal sparse attention (MLA-style KV).

Sharding: tensor-parallel over heads. 16 heads / 8 cores = 2 heads per core,
one KV head per core-pair. Each core computes q/k/v projections from the full
(transposed) x, RoPE, sparse attention for its 2 heads, and a partial output
projection; the host sums the 8 partial outputs.

Sparsity structure (T=4096, BLOCK=128, WINDOW=512, GLOBAL_EVERY=64):
for query block b, visible keys are blocks b-4..b (block b-4 masked by a fixed
triangular+global pattern) plus "global" columns j%64==0 with j < 128*(b-4).

All matmuls run as float32r (TF32-like, ~1.5e-4 rel err, full PE speed).
Scores are computed transposed ([k, q] layout) so probabilities feed the PV
and output-projection matmuls with no transposes. Softmax denominators are
computed with ones-matmul partition reductions accumulated in PSUM; the
reciprocal is broadcast across partitions on GPSIMD.
"""

import numpy as np

N_CORES = 8
T = 4096
C = 2048
L = 512
H = 16
KVH = 4
HD = 128
BLOCK = 128
WINDOW = 512
GLOBAL_EVERY = 64
ROPE_THETA = 10000.0

QTW = 512            # query tile width (4 blocks)
NQT = T // QTW       # 8
NKT = C // 128       # 16 contraction tiles for projections
NNT = T // 512       # 8 t-tiles for projections
NG = T // GLOBAL_EVERY  # 64 global columns

_CACHE = {}


def _build_module():
    import concourse.bacc as bacc
    import concourse.mybir as mybir
    import concourse.tile as tile
    from contextlib import ExitStack

    F32 = mybir.dt.float32
    F32R = mybir.dt.float32r
    EXP = mybir.ActivationFunctionType.Exp

    nc = bacc.Bacc("TRN2", target_bir_lowering=False, debug=False,
                   num_devices=N_CORES)

    xt = nc.dram_tensor("xt", [C, T], F32R, kind="ExternalInput")
    wq = nc.dram_tensor("wq", [C, 2 * HD], F32R, kind="ExternalInput")
    wk = nc.dram_tensor("wk", [C, HD], F32R, kind="ExternalInput")
    wv = nc.dram_tensor("wv", [C, HD], F32R, kind="ExternalInput")
    wo = nc.dram_tensor("wo", [2 * HD, C], F32R, kind="ExternalInput")
    cosd = nc.dram_tensor("cosd", [HD, T], F32, kind="ExternalInput")
    sind = nc.dram_tensor("sind", [HD, T], F32, kind="ExternalInput")  # sign-folded
    maskt = nc.dram_tensor("maskt", [128, 128], F32, kind="ExternalInput")
    maskg = nc.dram_tensor("maskg", [NG, T], F32, kind="ExternalInput")
    onesd = nc.dram_tensor("onesd", [128, 1], F32R, kind="ExternalInput")
    identd = nc.dram_tensor("identd", [128, 128], F32R, kind="ExternalInput")
    out = nc.dram_tensor("out", [T, C], F32, kind="ExternalOutput")

    scale = 1.0 / np.sqrt(HD)

    with tile.TileContext(nc) as tc, ExitStack() as ctx:
        res = ctx.enter_context(tc.tile_pool(name="res", bufs=1))
        qT = [res.tile([128, T], F32R, tag=f"qT{h}", name=f"qT{h}") for h in range(2)]
        kT = res.tile([128, T], F32R, tag="kT")
        vN = res.tile([128, T], F32R, tag="vN")
        kG = res.tile([128, NG], F32R, tag="kG")
        vG = res.tile([64, 128], F32R, tag="vG")
        mT = res.tile([128, 128], F32, tag="mT")
        mG = res.tile([NG, T], F32, tag="mG")
        ones = res.tile([128, 1], F32R, tag="ones")
        ident = res.tile([128, 128], F32R, tag="ident")
        wo_sb = res.tile([128, 2 * C], F32R, tag="wo_sb")

        nc.sync.dma_start(mT[:], maskt[:])
        nc.sync.dma_start(mG[:], maskg[:])
        nc.sync.dma_start(ones[:], onesd[:])
        nc.sync.dma_start(ident[:], identd[:])
        for i in range(2):
            nc.sync.dma_start(wo_sb[:, i * C:(i + 1) * C], wo[i * 128:(i + 1) * 128, :])

        # ---------------- Phase 1: projections + RoPE -----------------
        with ExitStack() as p1:
            wpool = p1.enter_context(tc.tile_pool(name="wpool", bufs=1))
            wq_sb = wpool.tile([128, NKT * 256], F32R, tag="wq_sb")
            wk_sb = wpool.tile([128, NKT * 128], F32R, tag="wk_sb")
            wv_sb = wpool.tile([128, NKT * 128], F32R, tag="wv_sb")
            cos_sb = wpool.tile([128, T], F32, tag="cos_sb")
            sin_sb = wpool.tile([128, T], F32, tag="sin_sb")
            vT = wpool.tile([128, T], F32R, tag="vT")
            for kt in range(NKT):
                nc.sync.dma_start(wq_sb[:, kt * 256:(kt + 1) * 256],
                                  wq[kt * 128:(kt + 1) * 128, :])
                nc.sync.dma_start(wk_sb[:, kt * 128:(kt + 1) * 128],
                                  wk[kt * 128:(kt + 1) * 128, :])
                nc.sync.dma_start(wv_sb[:, kt * 128:(kt + 1) * 128],
                                  wv[kt * 128:(kt + 1) * 128, :])
            nc.sync.dma_start(cos_sb[:], cosd[:])
            nc.sync.dma_start(sin_sb[:], sind[:])

            xpool = p1.enter_context(tc.tile_pool(name="xpool", bufs=3))
            swp = p1.enter_context(tc.tile_pool(name="swp", bufs=2))
            tmpp = p1.enter_context(tc.tile_pool(name="tmpp", bufs=2))
            pjps = p1.enter_context(tc.tile_pool(name="pjps", bufs=1, space="PSUM"))

            for nt in range(NNT):
                ts = slice(nt * 512, (nt + 1) * 512)
                ps = [pjps.tile([128, 512], F32, tag=f"pj{i}", name=f"pj{i}") for i in range(4)]
                for kt in range(NKT):
                    xtile = xpool.tile([128, 512], F32R, tag="xtile")
                    nc.sync.dma_start(xtile[:], xt[kt * 128:(kt + 1) * 128, ts])
                    st = kt == 0
                    sp = kt == NKT - 1
                    nc.tensor.matmul(ps[0][:], wq_sb[:, kt * 256:kt * 256 + 128],
                                     xtile[:], start=st, stop=sp)
                    nc.tensor.matmul(ps[1][:], wq_sb[:, kt * 256 + 128:kt * 256 + 256],
                                     xtile[:], start=st, stop=sp)
                    nc.tensor.matmul(ps[2][:], wk_sb[:, kt * 128:(kt + 1) * 128],
                                     xtile[:], start=st, stop=sp)
                    nc.tensor.matmul(ps[3][:], wv_sb[:, kt * 128:(kt + 1) * 128],
                                     xtile[:], start=st, stop=sp)
                # RoPE for q0, q1, k (DMA cannot read PSUM: ACT-copy first)
                for i, dest in enumerate((qT[0], qT[1], kT)):
                    qsb = swp.tile([128, 512], F32, tag="qsb")
                    nc.scalar.copy(qsb[:], ps[i][:])
                    sw = swp.tile([128, 512], F32, tag="sw")
                    nc.sync.dma_start(sw[0:64, :], qsb[64:128, :])
                    nc.sync.dma_start(sw[64:128, :], qsb[0:64, :])
                    ta = tmpp.tile([128, 512], F32, tag="ta")
                    nc.vector.tensor_mul(ta[:], ps[i][:], cos_sb[:, ts])
                    tb = tmpp.tile([128, 512], F32, tag="tb")
                    nc.vector.tensor_mul(tb[:], sw[:], sin_sb[:, ts])
                    nc.vector.tensor_add(dest[:, ts], ta[:], tb[:])
                nc.vector.tensor_copy(vT[:, ts], ps[3][:])

            # ------------- Phase 2: v transpose + global K/V -------------
            trps = p1.enter_context(tc.tile_pool(name="trps", bufs=2, space="PSUM"))
            for j in range(T // 128):
                tp = trps.tile([128, 128], F32R, tag="tp")
                nc.tensor.transpose(tp[:], vT[:, j * 128:(j + 1) * 128], ident[:])
                nc.vector.tensor_copy(vN[:, j * 128:(j + 1) * 128], tp[:])
            nc.vector.tensor_copy(kG[:], kT[:][:, 0:T:GLOBAL_EVERY])
            vGT = wpool.tile([128, NG], F32R, tag="vGT")
            nc.vector.tensor_copy(vGT[:], vT[:][:, 0:T:GLOBAL_EVERY])
            tp2 = trps.tile([64, 128], F32R, tag="tp")
            nc.tensor.transpose(tp2[:], vGT[:, 0:64], ident[:])
            nc.vector.tensor_copy(vG[:], tp2[:])

        # ------------- Phase 3: attention + output projection -------------
        spool = ctx.enter_context(tc.tile_pool(name="spool", bufs=2, space="PSUM"))
        ypool = ctx.enter_context(tc.tile_pool(name="ypool", bufs=1, space="PSUM"))
        dpool = ctx.enter_context(tc.tile_pool(name="dpool", bufs=1, space="PSUM"))
        opool = ctx.enter_context(tc.tile_pool(name="opool", bufs=2, space="PSUM"))
        ppool = ctx.enter_context(tc.tile_pool(name="ppool", bufs=3))
        ynp = ctx.enter_context(tc.tile_pool(name="ynp", bufs=2))
        recp = ctx.enter_context(tc.tile_pool(name="recp", bufs=2))
        rbcp = ctx.enter_context(tc.tile_pool(name="rbcp", bufs=2))
        obp = ctx.enter_context(tc.tile_pool(name="obp", bufs=3))

        for qt in range(NQT):
            b0 = 4 * qt
            qs0 = qt * QTW
            ynorm = []
            for h in range(2):
                # items: (kb, qoff, width, tri_sub) — s/p tiles cover q columns
                # [qoff, qoff+width) of this query tile; tri_sub is the 128-wide
                # sub-block (within the covered range) to multiply by maskT.
                # full-width diagonal block FIRST: its start=True matmul
                # zeroes the entire PSUM bank, so later partial-width matmuls
                # accumulate into well-defined zeros.
                items = [(b0, 0, 512, None)]
                if qt == 0:
                    for j in range(3):
                        items.append((j + 1, (j + 1) * 128, (3 - j) * 128, None))
                    use_glob = False
                else:
                    for j in range(4):
                        items.append((b0 - 4 + j, 0, (j + 1) * 128, j))
                    for j in range(3):
                        items.append((b0 + 1 + j, (j + 1) * 128, (3 - j) * 128, None))
                    use_glob = True

                y_ps = ypool.tile([128, QTW], F32, tag=f"y{h}")
                d_ps = dpool.tile([1, QTW], F32, tag=f"d{h}")
                n_items = len(items) + (1 if use_glob else 0)

                s_tiles = [None] * n_items
                p_tiles = [None] * n_items

                def emit_qk(ii):
                    if ii < len(items):
                        kb, qoff, w, _ = items[ii]
                        s = spool.tile([128, QTW], F32, tag="s")
                        nc.tensor.matmul(
                            s[:, :w], kT[:, kb * 128:(kb + 1) * 128],
                            qT[h][:, qs0 + qoff:qs0 + qoff + w],
                            start=True, stop=True)
                        s_tiles[ii] = s
                    else:  # globals
                        s = spool.tile([128, QTW], F32, tag="s")
                        nc.tensor.matmul(s[:64, :], kG[:, :], qT[h][:, qs0:qs0 + QTW],
                                         start=True, stop=True)
                        s_tiles[ii] = s

                def emit_rest(ii):
                    first = ii == 0
                    last = ii == n_items - 1
                    if ii < len(items):
                        kb, qoff, w, tri = items[ii]
                        s = s_tiles[ii]
                        p = ppool.tile([128, QTW], F32R, tag="p")
                        nc.scalar.activation(p[:, :w], s[:, :w], EXP, scale=scale)
                        if tri is not None:
                            nc.vector.tensor_mul(p[:, tri * 128:(tri + 1) * 128],
                                                 p[:, tri * 128:(tri + 1) * 128], mT[:])
                        nc.tensor.matmul(y_ps[:, qoff:qoff + w],
                                         vN[:, kb * 128:(kb + 1) * 128], p[:, :w],
                                         start=first, stop=last)
                        nc.tensor.matmul(d_ps[:, qoff:qoff + w], ones[:, :], p[:, :w],
                                         start=first, stop=last)
                        p_tiles[ii] = p
                    else:
                        s = s_tiles[ii]
                        p = ppool.tile([128, QTW], F32R, tag="p")
                        nc.scalar.activation(p[:64, :], s[:64, :], EXP, scale=scale)
                        nc.vector.tensor_mul(p[:64, :], p[:64, :],
                                             mG[:, qs0:qs0 + QTW])
                        nc.tensor.matmul(y_ps[:, :], vG[:, :], p[:64, :],
                                         start=first, stop=last)
                        nc.tensor.matmul(d_ps[:, :], ones[:64, :], p[:64, :],
                                         start=first, stop=last)
                        p_tiles[ii] = p

                # lag-1 software pipeline: QK runs 1 item ahead of PV/denom
                emit_qk(0)
                for ii in range(n_items):
                    if ii + 1 < n_items:
                        emit_qk(ii + 1)
                    emit_rest(ii)

                rec = recp.tile([1, QTW], F32, tag="rec")
                nc.vector.reciprocal(rec[:], d_ps[:])
                rbc = rbcp.tile([128, QTW], F32, tag="rbc")
                nc.gpsimd.partition_broadcast(rbc[:], rec[:])
                yn = ynp.tile([128, QTW], F32R, tag=f"yn{h}")
                nc.vector.tensor_mul(yn[:], y_ps[:], rbc[:])
                ynorm.append(yn)

            # output projection for this query tile
            for qs in range(4):
                rows = slice(qs0 + qs * 128, qs0 + (qs + 1) * 128)
                for n in range(4):
                    o_ps = opool.tile([128, 512], F32, tag="o")
                    nc.tensor.matmul(o_ps[:], ynorm[0][:, qs * 128:(qs + 1) * 128],
                                     wo_sb[:, n * 512:n * 512 + 512],
                                     start=True, stop=False)
                    nc.tensor.matmul(o_ps[:], ynorm[1][:, qs * 128:(qs + 1) * 128],
                                     wo_sb[:, C + n * 512:C + n * 512 + 512],
                                     start=False, stop=True)
                    ob = obp.tile([128, 512], F32, tag="ob")
                    if (qs * 4 + n) % 2 == 0:
                        nc.scalar.copy(ob[:], o_ps[:])
                    else:
                        nc.vector.tensor_copy(ob[:], o_ps[:])
                    nc.sync.dma_start(out[rows, n * 512:(n + 1) * 512], ob[:])

    nc.compile()
    return nc


def _host_inputs(x, w_q, w_kv_down, w_k_up, w_v_up, w_o):
    """Build the per-core input maps (host-side shard + precompute)."""
    x2 = np.ascontiguousarray(x.reshape(T, C).astype(np.float32))
    xt = np.ascontiguousarray(x2.T)

    # RoPE tables, [hd, t] layout, sign folded into sin for the swapped term
    freqs = 1.0 / (ROPE_THETA ** (np.arange(0, HD, 2, dtype=np.float64) / HD))
    emb = np.arange(T, dtype=np.float64)[:, None] * freqs[None, :]   # [T, 64]
    cos = np.concatenate([np.cos(emb), np.cos(emb)], axis=-1)        # [T, 128]
    sin = np.concatenate([np.sin(emb), np.sin(emb)], axis=-1)
    cosT = np.ascontiguousarray(cos.T.astype(np.float32))            # [128, T]
    sinS = sin.T.copy()
    sinS[:64, :] *= -1.0
    sinS = np.ascontiguousarray(sinS.astype(np.float32))

    # fixed triangular+global mask for the b-4 key block, [k_off, q_off]
    oi = np.arange(128)
    mT = ((oi[None, :] <= oi[:, None]) | (oi[:, None] % 64 == 0)).astype(np.float32)

    # global-column mask [g, q]: visible iff 64 g < 128 (q//128 - 4)
    g = np.arange(NG)
    qb = np.arange(T) // BLOCK
    mG = (64 * g[:, None] < 128 * (qb[None, :] - 4)).astype(np.float32)

    onesv = np.ones((128, 1), np.float32)
    ident = np.eye(128, dtype=np.float32)

    wk_f = (w_kv_down.astype(np.float64) @ w_k_up.astype(np.float64))  # [C, KVH*HD]
    wv_f = (w_kv_down.astype(np.float64) @ w_v_up.astype(np.float64))

    in_maps = []
    for c in range(N_CORES):
        h0 = 2 * c
        kv = h0 // (H // KVH)
        wq_c = np.ascontiguousarray(
            w_q[:, h0 * HD:(h0 + 2) * HD].astype(np.float32))
        wk_c = np.ascontiguousarray(
            wk_f[:, kv * HD:(kv + 1) * HD].astype(np.float32))
        wv_c = np.ascontiguousarray(
            wv_f[:, kv * HD:(kv + 1) * HD].astype(np.float32))
        wo_c = np.ascontiguousarray(
            w_o[h0 * HD:(h0 + 2) * HD, :].astype(np.float32))
        in_maps.append({
            "xt": xt, "wq": wq_c, "wk": wk_c, "wv": wv_c, "wo": wo_c,
            "cosd": cosT, "sind": sinS, "maskt": mT, "maskg": mG,
            "onesd": onesv, "identd": ident,
        })
    return in_maps


def _get_module():
    if "nc" not in _CACHE:
        _CACHE["nc"] = _build_module()
    return _CACHE["nc"]


def kernel(x, w_q, w_kv_down, w_k_up, w_v_up, w_o):
    from concourse.bass_utils import run_bass_kernel_spmd

    nc = _get_module()
    in_maps = _host_inputs(x, w_q, w_kv_down, w_k_up, w_v_up, w_o)
    res = run_bass_kernel_spmd(nc, in_maps, list(range(N_CORES)))
    acc = np.zeros((T, C), np.float32)
    for c in range(N_CORES):
        acc += res.results[c]["out"]
    return acc.reshape(1, T, C)


# revision 13
# speedup vs baseline: 391.9079x; 391.9079x over previous
"""Trainium2 Bass kernel for block-causal sparse attention (MLA-style KV).

Sharding: tensor-parallel over heads. 16 heads / 8 cores = 2 heads per core,
one KV head per core-pair. Each core computes q/k/v projections from the full
(transposed) x, RoPE, sparse attention for its 2 heads, and a partial output
projection; the host sums the 8 partial outputs.

Sparsity structure (T=4096, BLOCK=128, WINDOW=512, GLOBAL_EVERY=64):
for query block b, visible keys are blocks b-4..b (block b-4 masked by a fixed
triangular+global pattern) plus "global" columns j%64==0 with j < 128*(b-4).

All matmuls run as float32r (TF32-like, ~1.5e-4 rel err, full PE speed).
Scores are computed transposed ([k, q] layout) so probabilities feed the PV
and output-projection matmuls with no transposes. Softmax denominators are
computed with ones-matmul partition reductions accumulated in PSUM; the
reciprocal is broadcast across partitions on GPSIMD.
"""

import numpy as np

N_CORES = 8
T = 4096
C = 2048
L = 512
H = 16
KVH = 4
HD = 128
BLOCK = 128
WINDOW = 512
GLOBAL_EVERY = 64
ROPE_THETA = 10000.0

QTW = 512            # query tile width (4 blocks)
NQT = T // QTW       # 8
NKT = C // 128       # 16 contraction tiles for projections
NNT = T // 512       # 8 t-tiles for projections
NG = T // GLOBAL_EVERY  # 64 global columns

_CACHE = {}


def _build_module():
    import concourse.bacc as bacc
    import concourse.mybir as mybir
    import concourse.tile as tile
    from contextlib import ExitStack

    F32 = mybir.dt.float32
    F32R = mybir.dt.float32r
    EXP = mybir.ActivationFunctionType.Exp

    nc = bacc.Bacc("TRN2", target_bir_lowering=False, debug=False,
                   num_devices=N_CORES)

    xt = nc.dram_tensor("xt", [C, T], F32R, kind="ExternalInput")
    wq = nc.dram_tensor("wq", [C, 2 * HD], F32R, kind="ExternalInput")
    wk = nc.dram_tensor("wk", [C, HD], F32R, kind="ExternalInput")
    wv = nc.dram_tensor("wv", [C, HD], F32R, kind="ExternalInput")
    wo = nc.dram_tensor("wo", [2 * HD, C], F32R, kind="ExternalInput")
    cosd = nc.dram_tensor("cosd", [HD, T], F32, kind="ExternalInput")
    sind = nc.dram_tensor("sind", [HD, T], F32, kind="ExternalInput")  # sign-folded
    maskt = nc.dram_tensor("maskt", [128, 128], F32, kind="ExternalInput")
    maskg = nc.dram_tensor("maskg", [NG, T], F32, kind="ExternalInput")
    onesd = nc.dram_tensor("onesd", [128, 1], F32R, kind="ExternalInput")
    identd = nc.dram_tensor("identd", [128, 128], F32R, kind="ExternalInput")
    out = nc.dram_tensor("out", [T, C], F32, kind="ExternalOutput")

    scale = 1.0 / np.sqrt(HD)

    with tile.TileContext(nc) as tc, ExitStack() as ctx:
        res = ctx.enter_context(tc.tile_pool(name="res", bufs=1))
        kT = res.tile([128, T], F32R, tag="kT")
        vN = res.tile([128, T], F32R, tag="vN")
        kG = res.tile([128, NG], F32R, tag="kG")
        vG = res.tile([64, 128], F32R, tag="vG")
        vGT = res.tile([128, NG], F32R, tag="vGT")
        mT = res.tile([128, 128], F32, tag="mT")
        mG = res.tile([NG, T], F32, tag="mG")
        ones = res.tile([128, 1], F32R, tag="ones")
        ident = res.tile([128, 128], F32R, tag="ident")
        wo_sb = res.tile([128, 2 * C], F32R, tag="wo_sb")
        wq_sb = res.tile([128, NKT * 256], F32R, tag="wq_sb")
        wk_sb = res.tile([128, NKT * 128], F32R, tag="wk_sb")
        wv_sb = res.tile([128, NKT * 128], F32R, tag="wv_sb")

        xpool = ctx.enter_context(tc.tile_pool(name="xpool", bufs=20))
        qlp = ctx.enter_context(tc.tile_pool(name="qlp", bufs=2))
        csp = ctx.enter_context(tc.tile_pool(name="csp", bufs=2))
        vtp = ctx.enter_context(tc.tile_pool(name="vtp", bufs=2))
        swp = ctx.enter_context(tc.tile_pool(name="swp", bufs=2))
        tmpp = ctx.enter_context(tc.tile_pool(name="tmpp", bufs=2))
        ppool = ctx.enter_context(tc.tile_pool(name="ppool", bufs=3))
        ynp = ctx.enter_context(tc.tile_pool(name="ynp", bufs=2))
        recp = ctx.enter_context(tc.tile_pool(name="recp", bufs=2))
        rbcp = ctx.enter_context(tc.tile_pool(name="rbcp", bufs=2))
        obp = ctx.enter_context(tc.tile_pool(name="obp", bufs=3))

        pjps = ctx.enter_context(tc.tile_pool(name="pjps", bufs=2, space="PSUM"))
        spool = ctx.enter_context(tc.tile_pool(name="spool", bufs=2, space="PSUM"))
        ypool = ctx.enter_context(tc.tile_pool(name="ypool", bufs=1, space="PSUM"))
        dpool = ctx.enter_context(tc.tile_pool(name="dpool", bufs=1, space="PSUM"))
        opool = ctx.enter_context(tc.tile_pool(name="opool", bufs=2, space="PSUM"))

        for it in range(NQT):
            nt = it
            b0 = 4 * it
            ts = slice(nt * 512, (nt + 1) * 512)
            qs0 = it * QTW

            # ---- projections for t-tile `nt` (q0, q1, k, v sequentially
            # through 2 PSUM slots; all 16 x-tiles stay resident in SBUF) ----
            xts = []
            cos_t = csp.tile([128, 512], F32, tag="cos")
            sin_t = csp.tile([128, 512], F32, tag="sin")
            nc.sync.dma_start(cos_t[:], cosd[:, ts])
            nc.sync.dma_start(sin_t[:], sind[:, ts])
            for kt in range(NKT):
                if it == 0:
                    nc.sync.dma_start(wq_sb[:, kt * 256:(kt + 1) * 256],
                                      wq[kt * 128:(kt + 1) * 128, :])
                    if kt == 0:
                        nc.gpsimd.dma_start(ident[:], identd[:])
                        nc.gpsimd.dma_start(mT[:], maskt[:])
                        nc.gpsimd.dma_start(ones[:], onesd[:])
                xtile = xpool.tile([128, 512], F32R, tag="xtile")
                nc.sync.dma_start(xtile[:], xt[kt * 128:(kt + 1) * 128, ts])
                xts.append(xtile)
            if it == 0:
                # k/v weights are first needed two PSUM passes later; keep
                # them off the q-projection critical DMA path
                for kt in range(NKT):
                    nc.sync.dma_start(wk_sb[:, kt * 128:(kt + 1) * 128],
                                      wk[kt * 128:(kt + 1) * 128, :])
                    nc.sync.dma_start(wv_sb[:, kt * 128:(kt + 1) * 128],
                                      wv[kt * 128:(kt + 1) * 128, :])

            qloc = [qlp.tile([128, 512], F32R, tag=f"ql{h}", name=f"ql{h}")
                    for h in range(2)]
            wslices = [
                lambda kt: wq_sb[:, kt * 256:kt * 256 + 128],
                lambda kt: wq_sb[:, kt * 256 + 128:kt * 256 + 256],
                lambda kt: wk_sb[:, kt * 128:(kt + 1) * 128],
                lambda kt: wv_sb[:, kt * 128:(kt + 1) * 128],
            ]
            vT_t = vtp.tile([128, 512], F32R, tag="vT")
            for i in range(4):
                pj = pjps.tile([128, 512], F32, tag="pj")
                for kt in range(NKT):
                    nc.tensor.matmul(pj[:], wslices[i](kt), xts[kt][:],
                                     start=(kt == 0), stop=(kt == NKT - 1))
                if i < 3:
                    # RoPE: dest = pj*cos + swap(pj)*sinS
                    dest = qloc[i][:] if i < 2 else kT[:, ts]
                    qsb = swp.tile([128, 512], F32, tag="qsb")
                    nc.scalar.copy(qsb[:], pj[:])
                    sw = swp.tile([128, 512], F32, tag="sw")
                    nc.gpsimd.dma_start(sw[0:64, :], qsb[64:128, :])
                    nc.gpsimd.dma_start(sw[64:128, :], qsb[0:64, :])
                    ta = tmpp.tile([128, 512], F32, tag="ta")
                    nc.vector.tensor_mul(ta[:], pj[:], cos_t[:])
                    tb = tmpp.tile([128, 512], F32, tag="tb")
                    nc.vector.tensor_mul(tb[:], sw[:], sin_t[:])
                    nc.vector.tensor_add(dest, ta[:], tb[:])
                else:
                    nc.vector.tensor_copy(vT_t[:], pj[:])

            if it == 0:
                nc.gpsimd.dma_start(mG[:], maskg[:])
                for i in range(2):
                    nc.sync.dma_start(wo_sb[:, i * C:(i + 1) * C],
                                      wo[i * 128:(i + 1) * 128, :])

            # ---- v transpose for this t-tile + incremental global K/V ----
            for j in range(4):
                blk = nt * 4 + j
                tp = spool.tile([128, 512], F32R, tag="s")  # reuse s slots
                nc.tensor.transpose(tp[:, :128], vT_t[:, j * 128:(j + 1) * 128],
                                    ident[:])
                nc.vector.tensor_copy(vN[:, blk * 128:(blk + 1) * 128],
                                      tp[:, :128])
            gsl = slice(nt * 8, (nt + 1) * 8)
            nc.vector.tensor_copy(kG[:, gsl], kT[:, ts][:, 0:512:GLOBAL_EVERY])
            nc.vector.tensor_copy(vGT[:, gsl], vT_t[:][:, 0:512:GLOBAL_EVERY])
            gw2 = 8 * (nt + 1)
            tpg = spool.tile([128, 512], F32R, tag="s")
            nc.tensor.transpose(tpg[:gw2, :128], vGT[:, :gw2], ident[:])
            nc.vector.tensor_copy(vG[:gw2, :], tpg[:gw2, :128])

            # ---- attention for query tile `it` (4 blocks b0..b0+3) ----
            gw = min(NG, 8 * it)   # written prefix of kG/vG; 0 for it=0
            ynorm = []
            for h in range(2):
                items = [(b0, 0, 512, None)]
                if it == 0:
                    for j in range(3):
                        items.append((j + 1, (j + 1) * 128, (3 - j) * 128, None))
                    use_glob = False
                else:
                    for j in range(4):
                        items.append((b0 - 4 + j, 0, (j + 1) * 128, j))
                    for j in range(3):
                        items.append((b0 + 1 + j, (j + 1) * 128, (3 - j) * 128, None))
                    use_glob = gw > 0

                y_ps = ypool.tile([128, QTW], F32, tag="y")
                d_ps = dpool.tile([1, QTW], F32, tag="d")
                n_items = len(items) + (1 if use_glob else 0)
                s_tiles = [None] * n_items

                def emit_qk(ii):
                    s = spool.tile([128, QTW], F32, tag="s")
                    if ii < len(items):
                        kb, qoff, w, _ = items[ii]
                        nc.tensor.matmul(
                            s[:, :w], kT[:, kb * 128:(kb + 1) * 128],
                            qloc[h][:, qoff:qoff + w],
                            start=True, stop=True)
                    else:
                        nc.tensor.matmul(s[:gw, :], kG[:, :gw], qloc[h][:],
                                         start=True, stop=True)
                    s_tiles[ii] = s

                def emit_rest(ii):
                    first = ii == 0
                    last = ii == n_items - 1
                    s = s_tiles[ii]
                    p = ppool.tile([128, QTW], F32R, tag="p")
                    if ii < len(items):
                        kb, qoff, w, tri = items[ii]
                        nc.scalar.activation(p[:, :w], s[:, :w], EXP, scale=scale)
                        if tri is not None:
                            nc.vector.tensor_mul(p[:, tri * 128:(tri + 1) * 128],
                                                 p[:, tri * 128:(tri + 1) * 128],
                                                 mT[:])
                        nc.tensor.matmul(y_ps[:, qoff:qoff + w],
                                         vN[:, kb * 128:(kb + 1) * 128], p[:, :w],
                                         start=first, stop=last)
                        nc.tensor.matmul(d_ps[:, qoff:qoff + w], ones[:, :],
                                         p[:, :w], start=first, stop=last)
                    else:
                        nc.scalar.activation(p[:gw, :], s[:gw, :], EXP, scale=scale)
                        nc.vector.tensor_mul(p[:gw, :], p[:gw, :],
                                             mG[:gw, qs0:qs0 + QTW])
                        nc.tensor.matmul(y_ps[:, :], vG[:gw, :], p[:gw, :],
                                         start=first, stop=last)
                        nc.tensor.matmul(d_ps[:, :], ones[:gw, :], p[:gw, :],
                                         start=first, stop=last)

                emit_qk(0)
                for ii in range(n_items):
                    if ii + 1 < n_items:
                        emit_qk(ii + 1)
                    emit_rest(ii)

                rec = recp.tile([1, QTW], F32, tag="rec")
                nc.vector.reciprocal(rec[:], d_ps[:])
                rbc = rbcp.tile([128, QTW], F32, tag="rbc")
                nc.gpsimd.partition_broadcast(rbc[:], rec[:])
                yn = ynp.tile([128, QTW], F32R, tag=f"yn{h}", name=f"yn{h}")
                nc.vector.tensor_mul(yn[:], y_ps[:], rbc[:])
                ynorm.append(yn)

            # ---- output projection for this query tile ----
            for qs in range(4):
                rows = slice(qs0 + qs * 128, qs0 + (qs + 1) * 128)
                for n in range(4):
                    o_ps = opool.tile([128, 512], F32, tag="o")
                    nc.tensor.matmul(o_ps[:], ynorm[0][:, qs * 128:(qs + 1) * 128],
                                     wo_sb[:, n * 512:n * 512 + 512],
                                     start=True, stop=False)
                    nc.tensor.matmul(o_ps[:], ynorm[1][:, qs * 128:(qs + 1) * 128],
                                     wo_sb[:, C + n * 512:C + n * 512 + 512],
                                     start=False, stop=True)
                    ob = obp.tile([128, 512], F32, tag="ob")
                    if (qs * 4 + n) % 2 == 0:
                        nc.scalar.copy(ob[:], o_ps[:])
                    else:
                        nc.vector.tensor_copy(ob[:], o_ps[:])
                    nc.sync.dma_start(out[rows, n * 512:(n + 1) * 512], ob[:])

    nc.compile()
    return nc


def _host_inputs(x, w_q, w_kv_down, w_k_up, w_v_up, w_o):
    """Build the per-core input maps (host-side shard + precompute)."""
    x = np.asarray(x)
    w_q = np.asarray(w_q)
    w_kv_down = np.asarray(w_kv_down)
    w_k_up = np.asarray(w_k_up)
    w_v_up = np.asarray(w_v_up)
    w_o = np.asarray(w_o)
    x2 = np.ascontiguousarray(x.reshape(T, C).astype(np.float32))
    xt = np.ascontiguousarray(x2.T)

    # RoPE tables, [hd, t] layout, sign folded into sin for the swapped term
    freqs = 1.0 / (ROPE_THETA ** (np.arange(0, HD, 2, dtype=np.float64) / HD))
    emb = np.arange(T, dtype=np.float64)[:, None] * freqs[None, :]   # [T, 64]
    cos = np.concatenate([np.cos(emb), np.cos(emb)], axis=-1)        # [T, 128]
    sin = np.concatenate([np.sin(emb), np.sin(emb)], axis=-1)
    cosT = np.ascontiguousarray(cos.T.astype(np.float32))            # [128, T]
    sinS = sin.T.copy()
    sinS[:64, :] *= -1.0
    sinS = np.ascontiguousarray(sinS.astype(np.float32))

    # fixed triangular+global mask for the b-4 key block, [k_off, q_off]
    oi = np.arange(128)
    mT = ((oi[None, :] <= oi[:, None]) | (oi[:, None] % 64 == 0)).astype(np.float32)

    # global-column mask [g, q]: visible iff 64 g < 128 (q//128 - 4)
    g = np.arange(NG)
    qb = np.arange(T) // BLOCK
    mG = (64 * g[:, None] < 128 * (qb[None, :] - 4)).astype(np.float32)

    onesv = np.ones((128, 1), np.float32)
    ident = np.eye(128, dtype=np.float32)

    wk_f = (w_kv_down.astype(np.float32) @ w_k_up.astype(np.float32))  # [C, KVH*HD]
    wv_f = (w_kv_down.astype(np.float32) @ w_v_up.astype(np.float32))

    in_maps = []
    for c in range(N_CORES):
        h0 = 2 * c
        kv = h0 // (H // KVH)
        wq_c = np.ascontiguousarray(
            w_q[:, h0 * HD:(h0 + 2) * HD].astype(np.float32))
        wk_c = np.ascontiguousarray(
            wk_f[:, kv * HD:(kv + 1) * HD].astype(np.float32))
        wv_c = np.ascontiguousarray(
            wv_f[:, kv * HD:(kv + 1) * HD].astype(np.float32))
        wo_c = np.ascontiguousarray(
            w_o[h0 * HD:(h0 + 2) * HD, :].astype(np.float32))
        in_maps.append({
            "xt": xt, "wq": wq_c, "wk": wk_c, "wv": wv_c, "wo": wo_c,
            "cosd": cosT, "sind": sinS, "maskt": mT, "maskg": mG,
            "onesd": onesv, "identd": ident,
        })
    return in_maps


def _get_module():
    if "nc" not in _CACHE:
        _CACHE["nc"] = _build_module()
    return _CACHE["nc"]


def kernel(x, w_q, w_kv_down, w_k_up, w_v_up, w_o):
    from concourse.bass_utils import run_bass_kernel_spmd

    nc = _get_module()
    in_maps = _host_inputs(x, w_q, w_kv_down, w_k_up, w_v_up, w_o)
    res = run_bass_kernel_spmd(nc, in_maps, list(range(N_CORES)))
    acc = np.zeros((T, C), np.float32)
    for c in range(N_CORES):
        acc += res.results[c]["out"]
    return acc.reshape(1, T, C)



# revision 17
# speedup vs baseline: 400.9961x; 1.0232x over previous
"""Trainium2 Bass kernel for block-causal sparse attention (MLA-style KV).

Sharding: tensor-parallel over heads. 16 heads / 8 cores = 2 heads per core,
one KV head per core-pair. Each core computes q/k/v projections from the full
(transposed) x, RoPE, sparse attention for its 2 heads, and a partial output
projection; the host sums the 8 partial outputs.

Sparsity structure (T=4096, BLOCK=128, WINDOW=512, GLOBAL_EVERY=64):
for query block b, visible keys are blocks b-4..b (block b-4 masked by a fixed
triangular+global pattern) plus "global" columns j%64==0 with j < 128*(b-4).

All matmuls run as float32r (TF32-like, ~1.5e-4 rel err, full PE speed).
Scores are computed transposed ([k, q] layout) so probabilities feed the PV
and output-projection matmuls with no transposes. Softmax denominators are
computed with ones-matmul partition reductions accumulated in PSUM; the
reciprocal is broadcast across partitions on GPSIMD.
"""

import numpy as np

N_CORES = 8
T = 4096
C = 2048
L = 512
H = 16
KVH = 4
HD = 128
BLOCK = 128
WINDOW = 512
GLOBAL_EVERY = 64
ROPE_THETA = 10000.0

QTW = 512            # query tile width (4 blocks)
NQT = T // QTW       # 8
NKT = C // 128       # 16 contraction tiles for projections
NNT = T // 512       # 8 t-tiles for projections
NG = T // GLOBAL_EVERY  # 64 global columns

_CACHE = {}


def _build_module():
    import concourse.bacc as bacc
    import concourse.mybir as mybir
    import concourse.tile as tile
    from contextlib import ExitStack

    F32 = mybir.dt.float32
    F32R = mybir.dt.float32r
    EXP = mybir.ActivationFunctionType.Exp

    nc = bacc.Bacc("TRN2", target_bir_lowering=False, debug=False,
                   num_devices=N_CORES)

    xt = nc.dram_tensor("xt", [C, T], F32R, kind="ExternalInput")
    wq = nc.dram_tensor("wq", [C, 2 * HD], F32R, kind="ExternalInput")
    wk = nc.dram_tensor("wk", [C, HD], F32R, kind="ExternalInput")
    wv = nc.dram_tensor("wv", [C, HD], F32R, kind="ExternalInput")
    wo = nc.dram_tensor("wo", [2 * HD, C], F32R, kind="ExternalInput")
    cosd = nc.dram_tensor("cosd", [HD, T], F32, kind="ExternalInput")
    sind = nc.dram_tensor("sind", [HD, T], F32, kind="ExternalInput")  # sign-folded
    maskt = nc.dram_tensor("maskt", [128, 128], F32, kind="ExternalInput")
    maskg = nc.dram_tensor("maskg", [NG, T], F32, kind="ExternalInput")
    onesd = nc.dram_tensor("onesd", [128, 1], F32R, kind="ExternalInput")
    identd = nc.dram_tensor("identd", [128, 128], F32R, kind="ExternalInput")
    out = nc.dram_tensor("out", [T, C], F32, kind="ExternalOutput")

    scale = 1.0 / np.sqrt(HD)

    with tile.TileContext(nc) as tc, ExitStack() as ctx:
        res = ctx.enter_context(tc.tile_pool(name="res", bufs=1))
        kT = res.tile([128, T], F32R, tag="kT")
        vN = res.tile([128, T], F32R, tag="vN")
        kG = res.tile([128, NG], F32R, tag="kG")
        vG = res.tile([64, 128], F32R, tag="vG")
        vGT = res.tile([128, NG], F32R, tag="vGT")
        mT = res.tile([128, 128], F32, tag="mT")
        mG = res.tile([NG, T], F32, tag="mG")
        ones = res.tile([128, 1], F32R, tag="ones")
        ident = res.tile([128, 128], F32R, tag="ident")
        wo_sb = res.tile([128, 2 * C], F32R, tag="wo_sb")
        wq_sb = res.tile([128, NKT * 256], F32R, tag="wq_sb")
        wk_sb = res.tile([128, NKT * 128], F32R, tag="wk_sb")
        wv_sb = res.tile([128, NKT * 128], F32R, tag="wv_sb")

        xpool = ctx.enter_context(tc.tile_pool(name="xpool", bufs=20))
        qlp = ctx.enter_context(tc.tile_pool(name="qlp", bufs=2))
        csp = ctx.enter_context(tc.tile_pool(name="csp", bufs=2))
        vtp = ctx.enter_context(tc.tile_pool(name="vtp", bufs=2))
        swp = ctx.enter_context(tc.tile_pool(name="swp", bufs=2))
        tmpp = ctx.enter_context(tc.tile_pool(name="tmpp", bufs=2))
        ppool = ctx.enter_context(tc.tile_pool(name="ppool", bufs=3))
        ynp = ctx.enter_context(tc.tile_pool(name="ynp", bufs=2))
        recp = ctx.enter_context(tc.tile_pool(name="recp", bufs=2))
        rbcp = ctx.enter_context(tc.tile_pool(name="rbcp", bufs=2))
        obp = ctx.enter_context(tc.tile_pool(name="obp", bufs=3))

        pjps = ctx.enter_context(tc.tile_pool(name="pjps", bufs=2, space="PSUM"))
        spool = ctx.enter_context(tc.tile_pool(name="spool", bufs=2, space="PSUM"))
        ypool = ctx.enter_context(tc.tile_pool(name="ypool", bufs=1, space="PSUM"))
        dpool = ctx.enter_context(tc.tile_pool(name="dpool", bufs=1, space="PSUM"))
        opool = ctx.enter_context(tc.tile_pool(name="opool", bufs=2, space="PSUM"))

        def emit_wo(ynorm, qs0):
            for qs in range(4):
                rows = slice(qs0 + qs * 128, qs0 + (qs + 1) * 128)
                for n in range(4):
                    o_ps = opool.tile([128, 512], F32, tag="o", name="o_ps")
                    nc.tensor.matmul(o_ps[:], ynorm[0][:, qs * 128:(qs + 1) * 128],
                                     wo_sb[:, n * 512:n * 512 + 512],
                                     start=True, stop=False)
                    nc.tensor.matmul(o_ps[:], ynorm[1][:, qs * 128:(qs + 1) * 128],
                                     wo_sb[:, C + n * 512:C + n * 512 + 512],
                                     start=False, stop=True)
                    ob = obp.tile([128, 512], F32, tag="ob", name="ob")
                    if (qs * 4 + n) % 2 == 0:
                        nc.scalar.copy(ob[:], o_ps[:])
                    else:
                        nc.vector.tensor_copy(ob[:], o_ps[:])
                    nc.sync.dma_start(out[rows, n * 512:(n + 1) * 512], ob[:])

        pending_wo = None
        for it in range(NQT):
            nt = it
            b0 = 4 * it
            ts = slice(nt * 512, (nt + 1) * 512)
            qs0 = it * QTW

            # ---- projections for t-tile `nt` (q0, q1, k, v sequentially
            # through 2 PSUM slots; all 16 x-tiles stay resident in SBUF) ----
            xts = []
            cos_t = csp.tile([128, 512], F32, tag="cos")
            sin_t = csp.tile([128, 512], F32, tag="sin")
            nc.sync.dma_start(cos_t[:], cosd[:, ts])
            nc.sync.dma_start(sin_t[:], sind[:, ts])
            for kt in range(NKT):
                if it == 0:
                    nc.sync.dma_start(wq_sb[:, kt * 256:(kt + 1) * 256],
                                      wq[kt * 128:(kt + 1) * 128, :])
                    if kt == 0:
                        nc.gpsimd.dma_start(ident[:], identd[:])
                        nc.gpsimd.dma_start(mT[:], maskt[:])
                        nc.gpsimd.dma_start(ones[:], onesd[:])
                xtile = xpool.tile([128, 512], F32R, tag="xtile")
                nc.sync.dma_start(xtile[:], xt[kt * 128:(kt + 1) * 128, ts])
                xts.append(xtile)
            if it == 0:
                # k/v weights are first needed two PSUM passes later; keep
                # them off the q-projection critical DMA path
                for kt in range(NKT):
                    nc.sync.dma_start(wk_sb[:, kt * 128:(kt + 1) * 128],
                                      wk[kt * 128:(kt + 1) * 128, :])
                    nc.sync.dma_start(wv_sb[:, kt * 128:(kt + 1) * 128],
                                      wv[kt * 128:(kt + 1) * 128, :])

            qloc = [qlp.tile([128, 512], F32R, tag=f"ql{h}", name=f"ql{h}")
                    for h in range(2)]
            wslices = [
                lambda kt: wq_sb[:, kt * 256:kt * 256 + 128],
                lambda kt: wq_sb[:, kt * 256 + 128:kt * 256 + 256],
                lambda kt: wk_sb[:, kt * 128:(kt + 1) * 128],
                lambda kt: wv_sb[:, kt * 128:(kt + 1) * 128],
            ]
            vT_t = vtp.tile([128, 512], F32R, tag="vT")
            for i in range(4):
                pj = pjps.tile([128, 512], F32, tag="pj")
                for kt in range(NKT):
                    nc.tensor.matmul(pj[:], wslices[i](kt), xts[kt][:],
                                     start=(kt == 0), stop=(kt == NKT - 1))
                if i < 3:
                    # RoPE: dest = pj*cos + swap(pj)*sinS
                    dest = qloc[i][:] if i < 2 else kT[:, ts]
                    qsb = swp.tile([128, 512], F32, tag="qsb")
                    nc.scalar.copy(qsb[:], pj[:])
                    sw = swp.tile([128, 512], F32, tag="sw")
                    nc.gpsimd.dma_start(sw[0:64, :], qsb[64:128, :])
                    nc.gpsimd.dma_start(sw[64:128, :], qsb[0:64, :])
                    ta = tmpp.tile([128, 512], F32, tag="ta")
                    nc.vector.tensor_mul(ta[:], pj[:], cos_t[:])
                    tb = tmpp.tile([128, 512], F32, tag="tb")
                    nc.vector.tensor_mul(tb[:], sw[:], sin_t[:])
                    nc.vector.tensor_add(dest, ta[:], tb[:])
                else:
                    nc.vector.tensor_copy(vT_t[:], pj[:])

            if it == 0:
                nc.gpsimd.dma_start(mG[:], maskg[:])
                for i in range(2):
                    nc.sync.dma_start(wo_sb[:, i * C:(i + 1) * C],
                                      wo[i * 128:(i + 1) * 128, :])

            # ---- v transpose for this t-tile + incremental global K/V ----
            for j in range(4):
                blk = nt * 4 + j
                tp = spool.tile([128, 512], F32R, tag="s")  # reuse s slots
                nc.tensor.transpose(tp[:, :128], vT_t[:, j * 128:(j + 1) * 128],
                                    ident[:])
                nc.vector.tensor_copy(vN[:, blk * 128:(blk + 1) * 128],
                                      tp[:, :128])
            gsl = slice(nt * 8, (nt + 1) * 8)
            nc.vector.tensor_copy(kG[:, gsl], kT[:, ts][:, 0:512:GLOBAL_EVERY])
            nc.vector.tensor_copy(vGT[:, gsl], vT_t[:][:, 0:512:GLOBAL_EVERY])
            gw2 = 8 * (nt + 1)
            tpg = spool.tile([128, 512], F32R, tag="s")
            nc.tensor.transpose(tpg[:gw2, :128], vGT[:, :gw2], ident[:])
            nc.vector.tensor_copy(vG[:gw2, :], tpg[:gw2, :128])

            # ---- attention for query tile `it` (4 blocks b0..b0+3) ----
            gw = min(NG, 8 * it)   # written prefix of kG/vG; 0 for it=0
            ynorm = []
            for h in range(2):
                if h == 1 and pending_wo is not None:
                    emit_wo(*pending_wo)
                    pending_wo = None
                items = [(b0, 0, 512, None)]
                if it == 0:
                    for j in range(3):
                        items.append((j + 1, (j + 1) * 128, (3 - j) * 128, None))
                    use_glob = False
                else:
                    for j in range(4):
                        items.append((b0 - 4 + j, 0, (j + 1) * 128, j))
                    for j in range(3):
                        items.append((b0 + 1 + j, (j + 1) * 128, (3 - j) * 128, None))
                    use_glob = gw > 0

                y_ps = ypool.tile([128, QTW], F32, tag="y")
                d_ps = dpool.tile([1, QTW], F32, tag="d")
                n_items = len(items) + (1 if use_glob else 0)
                s_tiles = [None] * n_items

                def emit_qk(ii):
                    s = spool.tile([128, QTW], F32, tag="s")
                    if ii < len(items):
                        kb, qoff, w, _ = items[ii]
                        nc.tensor.matmul(
                            s[:, :w], kT[:, kb * 128:(kb + 1) * 128],
                            qloc[h][:, qoff:qoff + w],
                            start=True, stop=True)
                    else:
                        nc.tensor.matmul(s[:gw, :], kG[:, :gw], qloc[h][:],
                                         start=True, stop=True)
                    s_tiles[ii] = s

                def emit_rest(ii):
                    first = ii == 0
                    last = ii == n_items - 1
                    s = s_tiles[ii]
                    p = ppool.tile([128, QTW], F32R, tag="p")
                    if ii < len(items):
                        kb, qoff, w, tri = items[ii]
                        nc.scalar.activation(p[:, :w], s[:, :w], EXP, scale=scale)
                        if tri is not None:
                            nc.vector.tensor_mul(p[:, tri * 128:(tri + 1) * 128],
                                                 p[:, tri * 128:(tri + 1) * 128],
                                                 mT[:])
                        nc.tensor.matmul(y_ps[:, qoff:qoff + w],
                                         vN[:, kb * 128:(kb + 1) * 128], p[:, :w],
                                         start=first, stop=last)
                        nc.tensor.matmul(d_ps[:, qoff:qoff + w], ones[:, :],
                                         p[:, :w], start=first, stop=last)
                    else:
                        nc.scalar.activation(p[:gw, :], s[:gw, :], EXP, scale=scale)
                        nc.vector.tensor_mul(p[:gw, :], p[:gw, :],
                                             mG[:gw, qs0:qs0 + QTW])
                        nc.tensor.matmul(y_ps[:, :], vG[:gw, :], p[:gw, :],
                                         start=first, stop=last)
                        nc.tensor.matmul(d_ps[:, :], ones[:gw, :], p[:gw, :],
                                         start=first, stop=last)

                emit_qk(0)
                for ii in range(n_items):
                    if ii + 1 < n_items:
                        emit_qk(ii + 1)
                    emit_rest(ii)

                rec = recp.tile([1, QTW], F32, tag="rec")
                nc.vector.reciprocal(rec[:], d_ps[:])
                rbc = rbcp.tile([128, QTW], F32, tag="rbc")
                nc.gpsimd.partition_broadcast(rbc[:], rec[:])
                yn = ynp.tile([128, QTW], F32R, tag=f"yn{h}", name=f"yn{h}")
                nc.vector.tensor_mul(yn[:], y_ps[:], rbc[:])
                ynorm.append(yn)

            # ---- output projection: deferred to overlap with the next
            # iteration's projection matmuls (hides the normalize latency) ----
            pending_wo = (ynorm, qs0)

        emit_wo(*pending_wo)

    nc.compile()
    return nc


def _host_inputs(x, w_q, w_kv_down, w_k_up, w_v_up, w_o):
    """Build the per-core input maps (host-side shard + precompute)."""
    x = np.asarray(x)
    w_q = np.asarray(w_q)
    w_kv_down = np.asarray(w_kv_down)
    w_k_up = np.asarray(w_k_up)
    w_v_up = np.asarray(w_v_up)
    w_o = np.asarray(w_o)
    x2 = np.ascontiguousarray(x.reshape(T, C).astype(np.float32))
    xt = np.ascontiguousarray(x2.T)

    # RoPE tables, [hd, t] layout, sign folded into sin for the swapped term
    freqs = 1.0 / (ROPE_THETA ** (np.arange(0, HD, 2, dtype=np.float64) / HD))
    emb = np.arange(T, dtype=np.float64)[:, None] * freqs[None, :]   # [T, 64]
    cos = np.concatenate([np.cos(emb), np.cos(emb)], axis=-1)        # [T, 128]
    sin = np.concatenate([np.sin(emb), np.sin(emb)], axis=-1)
    cosT = np.ascontiguousarray(cos.T.astype(np.float32))            # [128, T]
    sinS = sin.T.copy()
    sinS[:64, :] *= -1.0
    sinS = np.ascontiguousarray(sinS.astype(np.float32))

    # fixed triangular+global mask for the b-4 key block, [k_off, q_off]
    oi = np.arange(128)
    mT = ((oi[None, :] <= oi[:, None]) | (oi[:, None] % 64 == 0)).astype(np.float32)

    # global-column mask [g, q]: visible iff 64 g < 128 (q//128 - 4)
    g = np.arange(NG)
    qb = np.arange(T) // BLOCK
    mG = (64 * g[:, None] < 128 * (qb[None, :] - 4)).astype(np.float32)

    onesv = np.ones((128, 1), np.float32)
    ident = np.eye(128, dtype=np.float32)

    wk_f = (w_kv_down.astype(np.float32) @ w_k_up.astype(np.float32))  # [C, KVH*HD]
    wv_f = (w_kv_down.astype(np.float32) @ w_v_up.astype(np.float32))

    in_maps = []
    for c in range(N_CORES):
        h0 = 2 * c
        kv = h0 // (H // KVH)
        wq_c = np.ascontiguousarray(
            w_q[:, h0 * HD:(h0 + 2) * HD].astype(np.float32))
        wk_c = np.ascontiguousarray(
            wk_f[:, kv * HD:(kv + 1) * HD].astype(np.float32))
        wv_c = np.ascontiguousarray(
            wv_f[:, kv * HD:(kv + 1) * HD].astype(np.float32))
        wo_c = np.ascontiguousarray(
            w_o[h0 * HD:(h0 + 2) * HD, :].astype(np.float32))
        in_maps.append({
            "xt": xt, "wq": wq_c, "wk": wk_c, "wv": wv_c, "wo": wo_c,
            "cosd": cosT, "sind": sinS, "maskt": mT, "maskg": mG,
            "onesd": onesv, "identd": ident,
        })
    return in_maps


def _get_module():
    if "nc" not in _CACHE:
        _CACHE["nc"] = _build_module()
    return _CACHE["nc"]


def kernel(x, w_q, w_kv_down, w_k_up, w_v_up, w_o):
    from concourse.bass_utils import run_bass_kernel_spmd

    nc = _get_module()
    in_maps = _host_inputs(x, w_q, w_kv_down, w_k_up, w_v_up, w_o)
    res = run_bass_kernel_spmd(nc, in_maps, list(range(N_CORES)))
    acc = np.zeros((T, C), np.float32)
    for c in range(N_CORES):
        acc += res.results[c]["out"]
    return acc.reshape(1, T, C)



# revision 21
# speedup vs baseline: 401.5944x; 1.0015x over previous
"""Trainium2 Bass kernel for block-causal sparse attention (MLA-style KV).

Sharding: tensor-parallel over heads. 16 heads / 8 cores = 2 heads per core,
one KV head per core-pair. Each core computes q/k/v projections from the full
(transposed) x, RoPE, sparse attention for its 2 heads, and a partial output
projection; the host sums the 8 partial outputs.

Sparsity structure (T=4096, BLOCK=128, WINDOW=512, GLOBAL_EVERY=64):
for query block b, visible keys are blocks b-4..b (block b-4 masked by a fixed
triangular+global pattern) plus "global" columns j%64==0 with j < 128*(b-4).

All matmuls run as float32r (TF32-like, ~1.5e-4 rel err, full PE speed).
Scores are computed transposed ([k, q] layout) so probabilities feed the PV
and output-projection matmuls with no transposes. Softmax denominators are
computed with ones-matmul partition reductions accumulated in PSUM; the
reciprocal is broadcast across partitions on GPSIMD.
"""

import numpy as np

N_CORES = 8
T = 4096
C = 2048
L = 512
H = 16
KVH = 4
HD = 128
BLOCK = 128
WINDOW = 512
GLOBAL_EVERY = 64
ROPE_THETA = 10000.0

QTW = 512            # query tile width (4 blocks)
NQT = T // QTW       # 8
NKT = C // 128       # 16 contraction tiles for projections
NNT = T // 512       # 8 t-tiles for projections
NG = T // GLOBAL_EVERY  # 64 global columns

_CACHE = {}


def _build_module():
    import concourse.bacc as bacc
    import concourse.mybir as mybir
    import concourse.tile as tile
    from contextlib import ExitStack

    F32 = mybir.dt.float32
    F32R = mybir.dt.float32r
    EXP = mybir.ActivationFunctionType.Exp

    nc = bacc.Bacc("TRN2", target_bir_lowering=False, debug=False,
                   num_devices=N_CORES)

    xt = nc.dram_tensor("xt", [C, T], F32R, kind="ExternalInput")
    wq = nc.dram_tensor("wq", [C, 2 * HD], F32R, kind="ExternalInput")
    wk = nc.dram_tensor("wk", [C, HD], F32R, kind="ExternalInput")
    wv = nc.dram_tensor("wv", [C, HD], F32R, kind="ExternalInput")
    wo = nc.dram_tensor("wo", [2 * HD, C], F32R, kind="ExternalInput")
    cosd = nc.dram_tensor("cosd", [HD, T], F32, kind="ExternalInput")
    sind = nc.dram_tensor("sind", [HD, T], F32, kind="ExternalInput")  # sign-folded
    maskt = nc.dram_tensor("maskt", [128, 128], F32, kind="ExternalInput")
    maskg = nc.dram_tensor("maskg", [NG, T], mybir.dt.bfloat16, kind="ExternalInput")
    onesd = nc.dram_tensor("onesd", [128, 1], F32R, kind="ExternalInput")
    identd = nc.dram_tensor("identd", [128, 128], F32R, kind="ExternalInput")
    out = nc.dram_tensor("out", [T, C], F32, kind="ExternalOutput")

    scale = 1.0 / np.sqrt(HD)

    with tile.TileContext(nc) as tc, ExitStack() as ctx:
        res = ctx.enter_context(tc.tile_pool(name="res", bufs=1))
        kT = res.tile([128, T], F32R, tag="kT")
        vN = res.tile([128, T], F32R, tag="vN")
        kG = res.tile([128, NG], F32R, tag="kG")
        vG = res.tile([64, 128], F32R, tag="vG")
        vGT = res.tile([128, NG], F32R, tag="vGT")
        mT = res.tile([128, 128], F32, tag="mT")
        mG = res.tile([NG, T], mybir.dt.bfloat16, tag="mG")
        ones = res.tile([128, 1], F32R, tag="ones")
        ident = res.tile([128, 128], F32R, tag="ident")
        wo_sb = res.tile([128, 2 * C], F32R, tag="wo_sb")
        wq_sb = res.tile([128, NKT * 256], F32R, tag="wq_sb")
        wk_sb = res.tile([128, NKT * 128], F32R, tag="wk_sb")
        wv_sb = res.tile([128, NKT * 128], F32R, tag="wv_sb")

        xpool = ctx.enter_context(tc.tile_pool(name="xpool", bufs=22))
        qlp = ctx.enter_context(tc.tile_pool(name="qlp", bufs=2))
        csp = ctx.enter_context(tc.tile_pool(name="csp", bufs=2))
        vtp = ctx.enter_context(tc.tile_pool(name="vtp", bufs=2))
        swp = ctx.enter_context(tc.tile_pool(name="swp", bufs=2))
        tmpp = ctx.enter_context(tc.tile_pool(name="tmpp", bufs=2))
        ppool = ctx.enter_context(tc.tile_pool(name="ppool", bufs=3))
        ynp = ctx.enter_context(tc.tile_pool(name="ynp", bufs=2))
        recp = ctx.enter_context(tc.tile_pool(name="recp", bufs=2))
        rbcp = ctx.enter_context(tc.tile_pool(name="rbcp", bufs=2))
        obp = ctx.enter_context(tc.tile_pool(name="obp", bufs=3))

        pjps = ctx.enter_context(tc.tile_pool(name="pjps", bufs=2, space="PSUM"))
        spool = ctx.enter_context(tc.tile_pool(name="spool", bufs=2, space="PSUM"))
        ypool = ctx.enter_context(tc.tile_pool(name="ypool", bufs=1, space="PSUM"))
        dpool = ctx.enter_context(tc.tile_pool(name="dpool", bufs=1, space="PSUM"))
        opool = ctx.enter_context(tc.tile_pool(name="opool", bufs=2, space="PSUM"))

        def emit_wo(ynorm, qs0):
            for qs in range(4):
                rows = slice(qs0 + qs * 128, qs0 + (qs + 1) * 128)
                for n in range(4):
                    o_ps = opool.tile([128, 512], F32, tag="o", name="o_ps")
                    nc.tensor.matmul(o_ps[:], ynorm[0][:, qs * 128:(qs + 1) * 128],
                                     wo_sb[:, n * 512:n * 512 + 512],
                                     start=True, stop=False)
                    nc.tensor.matmul(o_ps[:], ynorm[1][:, qs * 128:(qs + 1) * 128],
                                     wo_sb[:, C + n * 512:C + n * 512 + 512],
                                     start=False, stop=True)
                    ob = obp.tile([128, 512], F32, tag="ob", name="ob")
                    if (qs * 4 + n) % 2 == 0:
                        nc.scalar.copy(ob[:], o_ps[:])
                    else:
                        nc.vector.tensor_copy(ob[:], o_ps[:])
                    nc.sync.dma_start(out[rows, n * 512:(n + 1) * 512], ob[:])

        pending_wo = None
        for it in range(NQT):
            nt = it
            b0 = 4 * it
            ts = slice(nt * 512, (nt + 1) * 512)
            qs0 = it * QTW

            # ---- projections for t-tile `nt` (q0, q1, k, v sequentially
            # through 2 PSUM slots; all 16 x-tiles stay resident in SBUF) ----
            xts = []
            cos_t = csp.tile([128, 512], F32, tag="cos")
            sin_t = csp.tile([128, 512], F32, tag="sin")
            nc.sync.dma_start(cos_t[:], cosd[:, ts])
            nc.sync.dma_start(sin_t[:], sind[:, ts])
            for kt in range(NKT):
                if it == 0:
                    nc.sync.dma_start(wq_sb[:, kt * 256:(kt + 1) * 256],
                                      wq[kt * 128:(kt + 1) * 128, :])
                    if kt == 0:
                        nc.gpsimd.dma_start(ident[:], identd[:])
                        nc.gpsimd.dma_start(mT[:], maskt[:])
                        nc.gpsimd.dma_start(ones[:], onesd[:])
                xtile = xpool.tile([128, 512], F32R, tag="xtile")
                nc.sync.dma_start(xtile[:], xt[kt * 128:(kt + 1) * 128, ts])
                xts.append(xtile)
            if it == 0:
                # k/v weights are first needed two PSUM passes later; keep
                # them off the q-projection critical DMA path
                for kt in range(NKT):
                    nc.sync.dma_start(wk_sb[:, kt * 128:(kt + 1) * 128],
                                      wk[kt * 128:(kt + 1) * 128, :])
                    nc.sync.dma_start(wv_sb[:, kt * 128:(kt + 1) * 128],
                                      wv[kt * 128:(kt + 1) * 128, :])

            qloc = [qlp.tile([128, 512], F32R, tag=f"ql{h}", name=f"ql{h}")
                    for h in range(2)]
            wslices = [
                lambda kt: wq_sb[:, kt * 256:kt * 256 + 128],
                lambda kt: wq_sb[:, kt * 256 + 128:kt * 256 + 256],
                lambda kt: wk_sb[:, kt * 128:(kt + 1) * 128],
                lambda kt: wv_sb[:, kt * 128:(kt + 1) * 128],
            ]
            vT_t = vtp.tile([128, 512], F32R, tag="vT")
            for i in range(4):
                pj = pjps.tile([128, 512], F32, tag="pj")
                for kt in range(NKT):
                    nc.tensor.matmul(pj[:], wslices[i](kt), xts[kt][:],
                                     start=(kt == 0), stop=(kt == NKT - 1))
                if i < 3:
                    # RoPE: dest = pj*cos + swap(pj)*sinS
                    dest = qloc[i][:] if i < 2 else kT[:, ts]
                    qsb = swp.tile([128, 512], F32, tag="qsb")
                    nc.scalar.copy(qsb[:], pj[:])
                    sw = swp.tile([128, 512], F32, tag="sw")
                    nc.gpsimd.dma_start(sw[0:64, :], qsb[64:128, :])
                    nc.gpsimd.dma_start(sw[64:128, :], qsb[0:64, :])
                    ta = tmpp.tile([128, 512], F32, tag="ta")
                    nc.vector.tensor_mul(ta[:], pj[:], cos_t[:])
                    tb = tmpp.tile([128, 512], F32, tag="tb")
                    nc.vector.tensor_mul(tb[:], sw[:], sin_t[:])
                    nc.vector.tensor_add(dest, ta[:], tb[:])
                else:
                    nc.vector.tensor_copy(vT_t[:], pj[:])

            if it == 0:
                nc.gpsimd.dma_start(mG[:], maskg[:])
                for i in range(2):
                    nc.sync.dma_start(wo_sb[:, i * C:(i + 1) * C],
                                      wo[i * 128:(i + 1) * 128, :])

            # ---- v transpose for this t-tile + incremental global K/V ----
            for j in range(4):
                blk = nt * 4 + j
                tp = spool.tile([128, 512], F32R, tag="s", name="tp")
                nc.tensor.transpose(tp[:, :128], vT_t[:, j * 128:(j + 1) * 128],
                                    ident[:])
                nc.vector.tensor_copy(vN[:, blk * 128:(blk + 1) * 128],
                                      tp[:, :128])
            gsl = slice(nt * 8, (nt + 1) * 8)
            nc.vector.tensor_copy(kG[:, gsl], kT[:, ts][:, 0:512:GLOBAL_EVERY])
            nc.vector.tensor_copy(vGT[:, gsl], vT_t[:][:, 0:512:GLOBAL_EVERY])
            gw2 = 8 * (nt + 1)
            tpg = spool.tile([128, 512], F32R, tag="s", name="tpg")
            nc.tensor.transpose(tpg[:gw2, :128], vGT[:, :gw2], ident[:])
            nc.vector.tensor_copy(vG[:gw2, :], tpg[:gw2, :128])

            # ---- attention for query tile `it` (4 blocks b0..b0+3) ----
            gw = min(NG, 8 * it)   # written prefix of kG/vG; 0 for it=0
            ynorm = []
            for h in range(2):
                if h == 1 and pending_wo is not None:
                    emit_wo(*pending_wo)
                    pending_wo = None
                items = [(b0, 0, 512, None)]
                if it == 0:
                    for j in range(3):
                        items.append((j + 1, (j + 1) * 128, (3 - j) * 128, None))
                    use_glob = False
                else:
                    for j in range(4):
                        items.append((b0 - 4 + j, 0, (j + 1) * 128, j))
                    for j in range(3):
                        items.append((b0 + 1 + j, (j + 1) * 128, (3 - j) * 128, None))
                    use_glob = gw > 0

                y_ps = ypool.tile([128, QTW], F32, tag="y")
                d_ps = dpool.tile([1, QTW], F32, tag="d")
                n_items = len(items) + (1 if use_glob else 0)
                s_tiles = [None] * n_items

                def emit_qk(ii):
                    s = spool.tile([128, QTW], F32, tag="s")
                    if ii < len(items):
                        kb, qoff, w, _ = items[ii]
                        nc.tensor.matmul(
                            s[:, :w], kT[:, kb * 128:(kb + 1) * 128],
                            qloc[h][:, qoff:qoff + w],
                            start=True, stop=True)
                    else:
                        nc.tensor.matmul(s[:gw, :], kG[:, :gw], qloc[h][:],
                                         start=True, stop=True)
                    s_tiles[ii] = s

                def emit_rest(ii):
                    first = ii == 0
                    last = ii == n_items - 1
                    s = s_tiles[ii]
                    p = ppool.tile([128, QTW], F32R, tag="p")
                    if ii < len(items):
                        kb, qoff, w, tri = items[ii]
                        nc.scalar.activation(p[:, :w], s[:, :w], EXP, scale=scale)
                        if tri is not None:
                            nc.vector.tensor_mul(p[:, tri * 128:(tri + 1) * 128],
                                                 p[:, tri * 128:(tri + 1) * 128],
                                                 mT[:])
                        nc.tensor.matmul(y_ps[:, qoff:qoff + w],
                                         vN[:, kb * 128:(kb + 1) * 128], p[:, :w],
                                         start=first, stop=last)
                        nc.tensor.matmul(d_ps[:, qoff:qoff + w], ones[:, :],
                                         p[:, :w], start=first, stop=last)
                    else:
                        nc.scalar.activation(p[:gw, :], s[:gw, :], EXP, scale=scale)
                        nc.vector.tensor_mul(p[:gw, :], p[:gw, :],
                                             mG[:gw, qs0:qs0 + QTW])
                        nc.tensor.matmul(y_ps[:, :], vG[:gw, :], p[:gw, :],
                                         start=first, stop=last)
                        nc.tensor.matmul(d_ps[:, :], ones[:gw, :], p[:gw, :],
                                         start=first, stop=last)

                emit_qk(0)
                for ii in range(n_items):
                    if ii + 1 < n_items:
                        emit_qk(ii + 1)
                    emit_rest(ii)

                rec = recp.tile([1, QTW], F32, tag="rec")
                nc.vector.reciprocal(rec[:], d_ps[:])
                rbc = rbcp.tile([128, QTW], F32, tag="rbc")
                nc.gpsimd.partition_broadcast(rbc[:], rec[:])
                yn = ynp.tile([128, QTW], F32R, tag=f"yn{h}", name=f"yn{h}")
                nc.vector.tensor_mul(yn[:], y_ps[:], rbc[:])
                ynorm.append(yn)

            # ---- output projection: deferred to overlap with the next
            # iteration's projection matmuls (hides the normalize latency) ----
            pending_wo = (ynorm, qs0)

        emit_wo(*pending_wo)

    nc.compile()
    return nc


def _host_inputs(x, w_q, w_kv_down, w_k_up, w_v_up, w_o):
    """Build the per-core input maps (host-side shard + precompute)."""
    x = np.asarray(x)
    w_q = np.asarray(w_q)
    w_kv_down = np.asarray(w_kv_down)
    w_k_up = np.asarray(w_k_up)
    w_v_up = np.asarray(w_v_up)
    w_o = np.asarray(w_o)
    x2 = np.ascontiguousarray(x.reshape(T, C).astype(np.float32))
    xt = np.ascontiguousarray(x2.T)

    # RoPE tables, [hd, t] layout, sign folded into sin for the swapped term
    freqs = 1.0 / (ROPE_THETA ** (np.arange(0, HD, 2, dtype=np.float64) / HD))
    emb = np.arange(T, dtype=np.float64)[:, None] * freqs[None, :]   # [T, 64]
    cos = np.concatenate([np.cos(emb), np.cos(emb)], axis=-1)        # [T, 128]
    sin = np.concatenate([np.sin(emb), np.sin(emb)], axis=-1)
    cosT = np.ascontiguousarray(cos.T.astype(np.float32))            # [128, T]
    sinS = sin.T.copy()
    sinS[:64, :] *= -1.0
    sinS = np.ascontiguousarray(sinS.astype(np.float32))

    # fixed triangular+global mask for the b-4 key block, [k_off, q_off]
    oi = np.arange(128)
    mT = ((oi[None, :] <= oi[:, None]) | (oi[:, None] % 64 == 0)).astype(np.float32)

    # global-column mask [g, q]: visible iff 64 g < 128 (q//128 - 4)
    g = np.arange(NG)
    qb = np.arange(T) // BLOCK
    import ml_dtypes
    mG = (64 * g[:, None] < 128 * (qb[None, :] - 4)).astype(ml_dtypes.bfloat16)

    onesv = np.ones((128, 1), np.float32)
    ident = np.eye(128, dtype=np.float32)

    wk_f = (w_kv_down.astype(np.float32) @ w_k_up.astype(np.float32))  # [C, KVH*HD]
    wv_f = (w_kv_down.astype(np.float32) @ w_v_up.astype(np.float32))

    in_maps = []
    for c in range(N_CORES):
        h0 = 2 * c
        kv = h0 // (H // KVH)
        wq_c = np.ascontiguousarray(
            w_q[:, h0 * HD:(h0 + 2) * HD].astype(np.float32))
        wk_c = np.ascontiguousarray(
            wk_f[:, kv * HD:(kv + 1) * HD].astype(np.float32))
        wv_c = np.ascontiguousarray(
            wv_f[:, kv * HD:(kv + 1) * HD].astype(np.float32))
        wo_c = np.ascontiguousarray(
            w_o[h0 * HD:(h0 + 2) * HD, :].astype(np.float32))
        in_maps.append({
            "xt": xt, "wq": wq_c, "wk": wk_c, "wv": wv_c, "wo": wo_c,
            "cosd": cosT, "sind": sinS, "maskt": mT, "maskg": mG,
            "onesd": onesv, "identd": ident,
        })
    return in_maps


def _get_module():
    if "nc" not in _CACHE:
        _CACHE["nc"] = _build_module()
    return _CACHE["nc"]


def kernel(x, w_q, w_kv_down, w_k_up, w_v_up, w_o):
    from concourse.bass_utils import run_bass_kernel_spmd

    nc = _get_module()
    in_maps = _host_inputs(x, w_q, w_kv_down, w_k_up, w_v_up, w_o)
    res = run_bass_kernel_spmd(nc, in_maps, list(range(N_CORES)))
    acc = np.zeros((T, C), np.float32)
    for c in range(N_CORES):
        acc += res.results[c]["out"]
    return acc.reshape(1, T, C)

